# revision 37
# baseline (speedup 1.0000x reference)
"""Trainium2 Bass kernel for ColaViT pre-attention QKV down-projection.

Computes gelu(hidden_states @ concat(w_q, w_k, w_v)) and splits into
(q_low, k_low, v_low), matching the fp32 jax reference.

Sharding: data-parallel on batch across 8 NeuronCores; each core owns
M=1576 token rows of the [12608, 768] x [768, 576] GEMM + exact Gelu.

v5 strategy (from v4 NTFF analysis):
- DMA_DIRECT2D trigger cost is ~0.7us FIXED per descriptor (1.5us for
  partial-partition descriptors). v4 spent ~14us of engine time on 20
  triggers; loads serialized behind a 0.73us/trigger rate on sync.
- All tiles are full 128 rows: the 40-row tail is folded into a 13th
  m-tile that overlaps the 12th by 88 rows (recompute is free - matmul
  cost depends only on the moving dim).
- w k-slices split across BOTH HWDGE rings (sync: w0/w2/w4 + x,
  scalar: w1/w3/w5) so trigger issue rate never gates w arrival; the
  x0+w wall is then pure HBM bandwidth (~1.1MB -> ~3.3us).
- Only 2 warmup matmuls (~1.2us) bridge the preamble->first-data gap;
  real stream starts ~9.3us vs ~11.2us in v4.
- x in 7 descriptors sized so each chunk lands before the PE needs it.
- Stores in 6 batches on the scalar ring; the last two are single
  m-tiles so the final store data lands ~1.2us after the last matmul.
- Tail warmup matmuls keep the PE warm into the NRT postamble.
- fp16 in/out; fp32 PSUM accumulate; exact-Gelu ACTIVATE per m-tile.
"""

import numpy as np

HIDDEN = 768
RANK = 192
N_OUT = 3 * RANK          # 576
B, S = 64, 197
N_CORES = 8
M_PER_CORE = B * S // N_CORES   # 1576
P = 128
K_TILES = HIDDEN // P     # 6
N_CHUNK = 288             # one n-half (psum bank holds 512 fp32)
N_TILES = 13              # 12 full + 1 overlapped (rows 1448..1575)
N_WARMUP_MM = 6
N_TAILWARM_MM = 8

# col offset (within the core's 1576 rows) of each m-tile
TILE_OFF = [128 * t for t in range(12)] + [M_PER_CORE - P]

# x load descriptors: (tile-index list, k_lo, k_hi)
X_CHUNKS = [([0], 0, 6), ([1], 0, 6), ([2], 0, 6),
            ([3, 4], 0, 6), ([5, 6, 7], 0, 6), ([8, 9, 10], 0, 6),
            ([11, 12], 0, 6)]
# y store batches: list of tile-index lists
Y_BATCH = [[0, 1, 2], [3, 4, 5], [6, 7, 8], [9, 10], [11], [12]]

_CACHE = {}


def _build_nc():
    from contextlib import ExitStack

    import concourse.bacc as bacc
    import concourse.mybir as mybir
    from concourse.tile import TileContext

    f32 = mybir.dt.float32
    f16 = mybir.dt.float16
    gelu = mybir.ActivationFunctionType.Gelu

    nc = bacc.Bacc("TRN2", target_bir_lowering=False, debug=False,
                   num_devices=N_CORES)

    w_dram = [nc.dram_tensor(f"w{k}", [P, N_OUT], f16,
                             kind="ExternalInput") for k in range(3)]
    w34_dram = nc.dram_tensor("w34", [P, 2 * N_OUT], f16,
                              kind="ExternalInput")
    w5_dram = nc.dram_tensor("w5", [P, N_OUT], f16, kind="ExternalInput")
    x_dram = [nc.dram_tensor(f"x{ci}", [P, (khi - klo) * P * len(ts)],
                             f16, kind="ExternalInput")
              for ci, (ts, klo, khi) in enumerate(X_CHUNKS)]
    y_dram = [nc.dram_tensor(f"y{bi}", [P, len(ts) * N_OUT], f16,
                             kind="ExternalOutput")
              for bi, ts in enumerate(Y_BATCH)]

    # map (tile index, k) -> (x chunk idx, k row in chunk, col offset)
    tile2chunk = {}
    for ci, (ts, klo, khi) in enumerate(X_CHUNKS):
        for j, t in enumerate(ts):
            for k in range(klo, khi):
                tile2chunk[(t, k)] = (ci, k - klo, j * P)
    tile2batch = {}
    for bi, ts in enumerate(Y_BATCH):
        for j, t in enumerate(ts):
            tile2batch[t] = (bi, j)

    with TileContext(nc) as tc, ExitStack() as ctx:
        sb = ctx.enter_context(tc.tile_pool(name="sb", bufs=1))
        pp = ctx.enter_context(tc.tile_pool(name="pp", bufs=3, space="PSUM"))

        # PE warm-up: zero tile memset on the (otherwise idle) vector
        # engine, then 2 big matmuls bridging preamble -> first data.
        zt = sb.tile([P, 520], f16, tag="zt", name="zt")
        nc.vector.memset(zt[:], 0.0)
        zps = pp.tile([8, 512], f32, tag="zps", name="zps", bufs=1)
        for _ in range(N_WARMUP_MM):
            nc.tensor.matmul(zps[:], zt[:, :8], zt[:, 8:520],
                             start=True, stop=True)

        wt = [sb.tile([P, N_OUT], f16, tag=f"w{k}", name=f"w{k}")
              for k in range(3)]
        wt34 = sb.tile([P, 2, N_OUT], f16, tag="w34", name="w34")
        wt5 = sb.tile([P, N_OUT], f16, tag="w5", name="w5")
        xt = [sb.tile([P, khi - klo, P * len(ts)], f16, tag=f"x{ci}",
                      name=f"x{ci}")
              for ci, (ts, klo, khi) in enumerate(X_CHUNKS)]

        def load_x(ci):
            a = X_CHUNKS[ci][2] - X_CHUNKS[ci][1]
            nc.sync.dma_start(xt[ci][:], x_dram[ci][:].rearrange(
                "p (a m) -> p a m", a=a))

        # A single HWDGE ring streams ~170-220B/ns, so the 1.08MB of
        # x0+w can't all flow through one ring before tile 0 needs it.
        # Split: sync ring carries x0, w0, w1, w2 (arriving ~0.6us
        # apart, matching tile-0's cold-clock k consumption) and the
        # later x chunks; the scalar ring concurrently carries k3-5 as
        # ONE wide-row descriptor whose ~2.8us first-desc latency +
        # transfer lands right when tile 0 reaches k3.
        load_x(0)
        nc.scalar.dma_start(wt34[:], w34_dram[:].rearrange(
            "p (a n) -> p a n", a=2))
        nc.gpsimd.dma_start(wt[0][:], w_dram[0][:])
        nc.gpsimd.dma_start(wt[1][:], w_dram[1][:])
        nc.gpsimd.dma_start(wt[2][:], w_dram[2][:])
        nc.gpsimd.dma_start(wt5[:], w5_dram[:])
        for ci in range(1, len(X_CHUNKS)):
            load_x(ci)

        ysb = [sb.tile([P, len(ts), N_OUT], f16, tag=f"ysb{bi}",
                       name=f"ysb{bi}")
               for bi, ts in enumerate(Y_BATCH)]

        for t in range(N_TILES):
            bi, bj = tile2batch[t]
            ps = pp.tile([P, 2, 512], f32, tag="ps", name=f"ps{t}")
            for k in range(K_TILES):
                ci, kr, coff = tile2chunk[(t, k)]
                for nj in range(2):
                    n0, n1 = nj * N_CHUNK, (nj + 1) * N_CHUNK
                    if k < 3:
                        w_ap = wt[k][:, n0:n1]
                    elif k < 5:
                        w_ap = wt34[:, k - 3, n0:n1]
                    else:
                        w_ap = wt5[:, n0:n1]
                    nc.tensor.matmul(
                        ps[:, nj, :N_CHUNK],
                        xt[ci][:, kr, coff:coff + P],
                        w_ap,
                        start=(k == 0),
                        stop=(k == K_TILES - 1),
                    )
            nc.scalar.activation(ysb[bi][:, bj, :], ps[:, :, :N_CHUNK],
                                 gelu)
            if bj == len(Y_BATCH[bi]) - 1:
                nc.sync.dma_start(
                    y_dram[bi][:].rearrange("p (a n) -> p a n",
                                            a=len(Y_BATCH[bi])),
                    ysb[bi][:, :, :])

        # keep PE/NX busy into the final barrier -> warm NRT postamble
        for _ in range(N_TAILWARM_MM):
            nc.tensor.matmul(zps[:], zt[:, :8], zt[:, 8:520],
                             start=True, stop=True)

    nc.compile()
    return nc


def _get_nc():
    if "nc" not in _CACHE:
        _CACHE["nc"] = _build_nc()
    return _CACHE["nc"]


def _make_in_maps(hidden_states, w_q, w_k, w_v):
    x = np.asarray(hidden_states, dtype=np.float32).reshape(B * S, HIDDEN)
    xT16 = np.ascontiguousarray(x.T).astype(np.float16)     # [768, 12608]
    wcat = np.concatenate(
        [np.asarray(w_q, np.float32), np.asarray(w_k, np.float32),
         np.asarray(w_v, np.float32)], axis=1).astype(np.float16)

    w34 = np.ascontiguousarray(
        np.stack([wcat[k * P:(k + 1) * P, :] for k in (3, 4)],
                 axis=1).reshape(P, 2 * N_OUT))
    w5 = np.ascontiguousarray(wcat[5 * P:6 * P, :])

    in_maps = []
    for c in range(N_CORES):
        base = c * M_PER_CORE
        m = {f"w{k}": np.ascontiguousarray(wcat[k * P:(k + 1) * P, :])
             for k in range(3)}
        m["w34"] = w34
        m["w5"] = w5
        for ci, (ts, klo, khi) in enumerate(X_CHUNKS):
            segs = []
            for t in ts:
                seg = xT16[:, base + TILE_OFF[t]:base + TILE_OFF[t] + P]
                segs.append(seg.reshape(K_TILES, P, P)[klo:khi]
                            .transpose(1, 0, 2))
            arr = np.concatenate(segs, axis=2)   # [P, khi-klo, csz]
            m[f"x{ci}"] = np.ascontiguousarray(
                arr.reshape(P, (khi - klo) * P * len(ts)))
        in_maps.append(m)
    return in_maps


def _postprocess(results):
    y_full = np.empty((B * S, N_OUT), dtype=np.float32)
    for c in range(N_CORES):
        base = c * M_PER_CORE
        res = results[c]
        for bi, ts in enumerate(Y_BATCH):
            buf = res[f"y{bi}"].reshape(P, len(ts), N_OUT)
            for j, t in enumerate(ts):
                off = base + TILE_OFF[t]
                y_full[off:off + P, :] = buf[:, j, :]
    y_full = y_full.reshape(B, S, N_OUT)
    q = np.ascontiguousarray(y_full[:, :, :RANK])
    k = np.ascontiguousarray(y_full[:, :, RANK:2 * RANK])
    v = np.ascontiguousarray(y_full[:, :, 2 * RANK:])
    return (q, k, v)


def kernel(hidden_states, w_q, w_k, w_v):
    from concourse.bass_utils import run_bass_kernel_spmd

    nc = _get_nc()
    in_maps = _make_in_maps(hidden_states, w_q, w_k, w_v)
    res = run_bass_kernel_spmd(nc, in_maps, list(range(N_CORES)))
    return _postprocess(res.results)


# revision 38
# speedup vs baseline: 1.0620x; 1.0620x over previous
"""Trainium2 Bass kernel for ColaViT pre-attention QKV down-projection.

Computes gelu(hidden_states @ concat(w_q, w_k, w_v)) and splits into
(q_low, k_low, v_low), matching the fp32 jax reference.

Sharding: data-parallel on batch across 8 NeuronCores; each core owns
M=1576 token rows of the [12608, 768] x [768, 576] GEMM + exact Gelu.

v5 strategy (from v4 NTFF analysis):
- DMA_DIRECT2D trigger cost is ~0.7us FIXED per descriptor (1.5us for
  partial-partition descriptors). v4 spent ~14us of engine time on 20
  triggers; loads serialized behind a 0.73us/trigger rate on sync.
- All tiles are full 128 rows: the 40-row tail is folded into a 13th
  m-tile that overlaps the 12th by 88 rows (recompute is free - matmul
  cost depends only on the moving dim).
- w k-slices split across BOTH HWDGE rings (sync: w0/w2/w4 + x,
  scalar: w1/w3/w5) so trigger issue rate never gates w arrival; the
  x0+w wall is then pure HBM bandwidth (~1.1MB -> ~3.3us).
- Only 2 warmup matmuls (~1.2us) bridge the preamble->first-data gap;
  real stream starts ~9.3us vs ~11.2us in v4.
- x in 7 descriptors sized so each chunk lands before the PE needs it.
- Stores in 6 batches on the scalar ring; the last two are single
  m-tiles so the final store data lands ~1.2us after the last matmul.
- Tail warmup matmuls keep the PE warm into the NRT postamble.
- fp16 in/out; fp32 PSUM accumulate; exact-Gelu ACTIVATE per m-tile.
"""

import numpy as np

HIDDEN = 768
RANK = 192
N_OUT = 3 * RANK          # 576
B, S = 64, 197
N_CORES = 8
M_PER_CORE = B * S // N_CORES   # 1576
P = 128
K_TILES = HIDDEN // P     # 6
N_CHUNK = 288             # one n-half (psum bank holds 512 fp32)
N_TILES = 13              # 12 full + 1 overlapped (rows 1448..1575)
N_WARMUP_MM = 6
N_TAILWARM_MM = 8

# col offset (within the core's 1576 rows) of each m-tile
TILE_OFF = [128 * t for t in range(12)] + [M_PER_CORE - P]

# x load descriptors: (tile-index list, k_lo, k_hi)
X_CHUNKS = [([0], 0, 6), ([1], 0, 6), ([2], 0, 6),
            ([3, 4], 0, 6), ([5, 6, 7], 0, 6), ([8, 9, 10], 0, 6),
            ([11, 12], 0, 6)]
# y store batches: list of tile-index lists
Y_BATCH = [[0, 1, 2], [3, 4, 5], [6, 7, 8], [9, 10], [11], [12]]

_CACHE = {}


def _build_nc():
    from contextlib import ExitStack

    import concourse.bacc as bacc
    import concourse.mybir as mybir
    from concourse.tile import TileContext

    f32 = mybir.dt.float32
    f16 = mybir.dt.float16
    gelu = mybir.ActivationFunctionType.Gelu

    nc = bacc.Bacc("TRN2", target_bir_lowering=False, debug=False,
                   num_devices=N_CORES)

    w_dram = [nc.dram_tensor(f"w{k}", [P, N_OUT], f16,
                             kind="ExternalInput") for k in range(3)]
    w345_dram = nc.dram_tensor("w345", [P, 3 * N_OUT], f16,
                               kind="ExternalInput")
    x_dram = [nc.dram_tensor(f"x{ci}", [P, (khi - klo) * P * len(ts)],
                             f16, kind="ExternalInput")
              for ci, (ts, klo, khi) in enumerate(X_CHUNKS)]
    y_dram = [nc.dram_tensor(f"y{bi}", [P, len(ts) * N_OUT], f16,
                             kind="ExternalOutput")
              for bi, ts in enumerate(Y_BATCH)]

    # map (tile index, k) -> (x chunk idx, k row in chunk, col offset)
    tile2chunk = {}
    for ci, (ts, klo, khi) in enumerate(X_CHUNKS):
        for j, t in enumerate(ts):
            for k in range(klo, khi):
                tile2chunk[(t, k)] = (ci, k - klo, j * P)
    tile2batch = {}
    for bi, ts in enumerate(Y_BATCH):
        for j, t in enumerate(ts):
            tile2batch[t] = (bi, j)

    with TileContext(nc) as tc, ExitStack() as ctx:
        sb = ctx.enter_context(tc.tile_pool(name="sb", bufs=1))
        pp = ctx.enter_context(tc.tile_pool(name="pp", bufs=3, space="PSUM"))

        # PE warm-up: zero tile memset on the (otherwise idle) vector
        # engine, then 2 big matmuls bridging preamble -> first data.
        zt = sb.tile([P, 520], f16, tag="zt", name="zt")
        nc.vector.memset(zt[:], 0.0)
        zps = pp.tile([8, 512], f32, tag="zps", name="zps", bufs=1)
        for _ in range(N_WARMUP_MM):
            nc.tensor.matmul(zps[:], zt[:, :8], zt[:, 8:520],
                             start=True, stop=True)

        wt = [sb.tile([P, N_OUT], f16, tag=f"w{k}", name=f"w{k}")
              for k in range(3)]
        wt345 = sb.tile([P, 3, N_OUT], f16, tag="w345", name="w345")
        xt = [sb.tile([P, khi - klo, P * len(ts)], f16, tag=f"x{ci}",
                      name=f"x{ci}")
              for ci, (ts, klo, khi) in enumerate(X_CHUNKS)]

        def load_x(ci):
            a = X_CHUNKS[ci][2] - X_CHUNKS[ci][1]
            nc.sync.dma_start(xt[ci][:], x_dram[ci][:].rearrange(
                "p (a m) -> p a m", a=a))

        # A single HWDGE ring streams ~170-220B/ns, so the 1.08MB of
        # x0+w can't all flow through one ring before tile 0 needs it.
        # Split: sync ring carries x0, w0, w1, w2 (arriving ~0.6us
        # apart, matching tile-0's cold-clock k consumption) and the
        # later x chunks; the scalar ring concurrently carries k3-5 as
        # ONE wide-row descriptor whose ~2.8us first-desc latency +
        # transfer lands right when tile 0 reaches k3.
        nc.sync.dma_start(wt[0][:], w_dram[0][:])
        nc.scalar.dma_start(wt345[:], w345_dram[:].rearrange(
            "p (a n) -> p a n", a=3))
        nc.gpsimd.dma_start(wt[1][:], w_dram[1][:])
        nc.gpsimd.dma_start(wt[2][:], w_dram[2][:])
        for ci in range(len(X_CHUNKS)):
            load_x(ci)

        ysb = [sb.tile([P, len(ts), N_OUT], f16, tag=f"ysb{bi}",
                       name=f"ysb{bi}")
               for bi, ts in enumerate(Y_BATCH)]

        for t in range(N_TILES):
            bi, bj = tile2batch[t]
            ps = pp.tile([P, 2, 512], f32, tag="ps", name=f"ps{t}")
            for k in range(K_TILES):
                ci, kr, coff = tile2chunk[(t, k)]
                for nj in range(2):
                    n0, n1 = nj * N_CHUNK, (nj + 1) * N_CHUNK
                    w_ap = (wt[k][:, n0:n1] if k < 3
                            else wt345[:, k - 3, n0:n1])
                    nc.tensor.matmul(
                        ps[:, nj, :N_CHUNK],
                        xt[ci][:, kr, coff:coff + P],
                        w_ap,
                        start=(k == 0),
                        stop=(k == K_TILES - 1),
                    )
            nc.scalar.activation(ysb[bi][:, bj, :], ps[:, :, :N_CHUNK],
                                 gelu)
            if bj == len(Y_BATCH[bi]) - 1:
                nc.sync.dma_start(
                    y_dram[bi][:].rearrange("p (a n) -> p a n",
                                            a=len(Y_BATCH[bi])),
                    ysb[bi][:, :, :])

        # keep PE/NX busy into the final barrier -> warm NRT postamble
        for _ in range(N_TAILWARM_MM):
            nc.tensor.matmul(zps[:], zt[:, :8], zt[:, 8:520],
                             start=True, stop=True)

    nc.compile()
    return nc


def _get_nc():
    if "nc" not in _CACHE:
        _CACHE["nc"] = _build_nc()
    return _CACHE["nc"]


def _make_in_maps(hidden_states, w_q, w_k, w_v):
    x = np.asarray(hidden_states, dtype=np.float32).reshape(B * S, HIDDEN)
    xT16 = np.ascontiguousarray(x.T).astype(np.float16)     # [768, 12608]
    wcat = np.concatenate(
        [np.asarray(w_q, np.float32), np.asarray(w_k, np.float32),
         np.asarray(w_v, np.float32)], axis=1).astype(np.float16)

    w345 = np.ascontiguousarray(
        np.stack([wcat[k * P:(k + 1) * P, :] for k in (3, 4, 5)],
                 axis=1).reshape(P, 3 * N_OUT))

    in_maps = []
    for c in range(N_CORES):
        base = c * M_PER_CORE
        m = {f"w{k}": np.ascontiguousarray(wcat[k * P:(k + 1) * P, :])
             for k in range(3)}
        m["w345"] = w345
        for ci, (ts, klo, khi) in enumerate(X_CHUNKS):
            segs = []
            for t in ts:
                seg = xT16[:, base + TILE_OFF[t]:base + TILE_OFF[t] + P]
                segs.append(seg.reshape(K_TILES, P, P)[klo:khi]
                            .transpose(1, 0, 2))
            arr = np.concatenate(segs, axis=2)   # [P, khi-klo, csz]
            m[f"x{ci}"] = np.ascontiguousarray(
                arr.reshape(P, (khi - klo) * P * len(ts)))
        in_maps.append(m)
    return in_maps


def _postprocess(results):
    y_full = np.empty((B * S, N_OUT), dtype=np.float32)
    for c in range(N_CORES):
        base = c * M_PER_CORE
        res = results[c]
        for bi, ts in enumerate(Y_BATCH):
            buf = res[f"y{bi}"].reshape(P, len(ts), N_OUT)
            for j, t in enumerate(ts):
                off = base + TILE_OFF[t]
                y_full[off:off + P, :] = buf[:, j, :]
    y_full = y_full.reshape(B, S, N_OUT)
    q = np.ascontiguousarray(y_full[:, :, :RANK])
    k = np.ascontiguousarray(y_full[:, :, RANK:2 * RANK])
    v = np.ascontiguousarray(y_full[:, :, 2 * RANK:])
    return (q, k, v)


def kernel(hidden_states, w_q, w_k, w_v):
    from concourse.bass_utils import run_bass_kernel_spmd

    nc = _get_nc()
    in_maps = _make_in_maps(hidden_states, w_q, w_k, w_v)
    res = run_bass_kernel_spmd(nc, in_maps, list(range(N_CORES)))
    return _postprocess(res.results)


# revision 44
# speedup vs baseline: 1.0721x; 1.0095x over previous
"""Trainium2 Bass kernel for ColaViT pre-attention QKV down-projection.

Computes gelu(hidden_states @ concat(w_q, w_k, w_v)) and splits into
(q_low, k_low, v_low), matching the fp32 jax reference.

Sharding: data-parallel on batch across 8 NeuronCores; each core owns
M=1576 token rows of the [12608, 768] x [768, 576] GEMM + exact Gelu.

v12 strategy (36.7us median; from v4..v11 NTFF trace analysis):
- All tiles are full 128 rows: the 40-row tail is folded into a 13th
  m-tile that overlaps the 12th by 88 rows (recompute is free - matmul
  cost depends only on the moving dim, and partial-partition DMA
  descriptors cost ~1.5us to trigger vs ~0.65us for 128-row ones).
- Steady stream runs at the PE roofline: 122.5ns per 288-col matmul
  (288/2.4GHz + 2.5ns NX), 1470ns per m-tile, LDWEIGHTS fully hidden.
- A single HWDGE ring streams only ~130-200B/ns descriptor-by-
  descriptor, so the ~1.1MB of w+x0 that tile 0 needs is spread over
  THREE DMA paths issued in parallel right after the preamble:
  sync ring (w0, then x in 7 chunks sized to consumption order),
  scalar ring (w3-5 as one wide-row descriptor; its ~2.8us first-desc
  latency lands just as tile 0 reaches k3), and gpsimd SWDGE (w1, w2).
  Tile 0 then runs with no k-slice stall and the whole stream holds
  the 1470ns cadence from the first tile.
- 6 warmup zero-matmuls (zt memset on the idle vector engine) bridge
  preamble -> first data with no PE gap: an idle gap >~1us resets the
  HAM clock ramp and costs ~3.5us of 1.2GHz streaming.
- Stores in 6 batches (3/3/3/2/1/1 m-tiles) on the sync ring; the last
  two are single m-tiles so the final store lands ~2us after the last
  matmul. 8 tail warmup matmuls (cheap, after the last real matmul).
- fp16 in/out; fp32 PSUM accumulate; exact-Gelu ACTIVATE per m-tile
  evicting both psum banks (740ns, well under the 1470ns tile cadence).
- Remaining fixed costs: ~6.5us NRT preamble, ~7us postamble (253
  semaphore resets split ~51/engine + drains + final barrier).
"""

import numpy as np

HIDDEN = 768
RANK = 192
N_OUT = 3 * RANK          # 576
B, S = 64, 197
N_CORES = 8
M_PER_CORE = B * S // N_CORES   # 1576
P = 128
K_TILES = HIDDEN // P     # 6
N_CHUNK = 288             # one n-half (psum bank holds 512 fp32)
N_TILES = 13              # 12 full + 1 overlapped (rows 1448..1575)
N_WARMUP_MM = 6
N_TAILWARM_MM = 8

# col offset (within the core's 1576 rows) of each m-tile
TILE_OFF = [128 * t for t in range(12)] + [M_PER_CORE - P]

# x load descriptors: (tile-index list, k_lo, k_hi). Tile 0 is split
# into k-halves so the first matmul's data lands ~0.5us earlier.
X_CHUNKS = [([0], 0, 3), ([0], 3, 6), ([1], 0, 6), ([2], 0, 6),
            ([3, 4], 0, 6), ([5, 6, 7], 0, 6), ([8, 9, 10], 0, 6),
            ([11, 12], 0, 6)]
# y store batches: list of tile-index lists (tile 12 handled apart:
# its two n-halves are activated + stored separately for a short tail)
Y_BATCH = [[0, 1, 2], [3, 4, 5], [6, 7, 8], [9, 10], [11]]

_CACHE = {}


def _build_nc():
    from contextlib import ExitStack

    import concourse.bacc as bacc
    import concourse.mybir as mybir
    from concourse.tile import TileContext

    f32 = mybir.dt.float32
    f16 = mybir.dt.float16
    gelu = mybir.ActivationFunctionType.Gelu

    nc = bacc.Bacc("TRN2", target_bir_lowering=False, debug=False,
                   num_devices=N_CORES)

    w_dram = [nc.dram_tensor(f"w{k}", [P, N_OUT], f16,
                             kind="ExternalInput") for k in range(3)]
    w345_dram = nc.dram_tensor("w345", [P, 3 * N_OUT], f16,
                               kind="ExternalInput")
    x_dram = [nc.dram_tensor(f"x{ci}", [P, (khi - klo) * P * len(ts)],
                             f16, kind="ExternalInput")
              for ci, (ts, klo, khi) in enumerate(X_CHUNKS)]
    y_dram = [nc.dram_tensor(f"y{bi}", [P, len(ts) * N_OUT], f16,
                             kind="ExternalOutput")
              for bi, ts in enumerate(Y_BATCH)]
    y12_dram = [nc.dram_tensor(f"y12{h}", [P, N_CHUNK], f16,
                               kind="ExternalOutput") for h in range(2)]

    # map (tile index, k) -> (x chunk idx, k row in chunk, col offset)
    tile2chunk = {}
    for ci, (ts, klo, khi) in enumerate(X_CHUNKS):
        for j, t in enumerate(ts):
            for k in range(klo, khi):
                tile2chunk[(t, k)] = (ci, k - klo, j * P)
    tile2batch = {}
    for bi, ts in enumerate(Y_BATCH):
        for j, t in enumerate(ts):
            tile2batch[t] = (bi, j)

    with TileContext(nc) as tc, ExitStack() as ctx:
        sb = ctx.enter_context(tc.tile_pool(name="sb", bufs=1))
        pp = ctx.enter_context(tc.tile_pool(name="pp", bufs=3, space="PSUM"))

        # PE warm-up: zero tile memset on the (otherwise idle) vector
        # engine, then 2 big matmuls bridging preamble -> first data.
        zt = sb.tile([P, 520], f16, tag="zt", name="zt")
        nc.vector.memset(zt[:], 0.0)
        zps = pp.tile([8, 512], f32, tag="zps", name="zps", bufs=1)
        for _ in range(N_WARMUP_MM):
            nc.tensor.matmul(zps[:], zt[:, :8], zt[:, 8:520],
                             start=True, stop=True)

        wt = [sb.tile([P, N_OUT], f16, tag=f"w{k}", name=f"w{k}")
              for k in range(3)]
        wt345 = sb.tile([P, 3, N_OUT], f16, tag="w345", name="w345")
        xt = [sb.tile([P, khi - klo, P * len(ts)], f16, tag=f"x{ci}",
                      name=f"x{ci}")
              for ci, (ts, klo, khi) in enumerate(X_CHUNKS)]

        def load_x(ci):
            a = X_CHUNKS[ci][2] - X_CHUNKS[ci][1]
            nc.sync.dma_start(xt[ci][:], x_dram[ci][:].rearrange(
                "p (a m) -> p a m", a=a))

        # A single HWDGE ring streams ~170-220B/ns, so the 1.08MB of
        # x0+w can't all flow through one ring before tile 0 needs it.
        # Split: sync ring carries x0, w0, w1, w2 (arriving ~0.6us
        # apart, matching tile-0's cold-clock k consumption) and the
        # later x chunks; the scalar ring concurrently carries k3-5 as
        # ONE wide-row descriptor whose ~2.8us first-desc latency +
        # transfer lands right when tile 0 reaches k3.
        load_x(0)
        nc.scalar.dma_start(wt345[:], w345_dram[:].rearrange(
            "p (a n) -> p a n", a=3))
        nc.gpsimd.dma_start(wt[1][:], w_dram[1][:])
        nc.gpsimd.dma_start(wt[2][:], w_dram[2][:])
        nc.sync.dma_start(wt[0][:], w_dram[0][:])
        for ci in range(1, len(X_CHUNKS)):
            load_x(ci)

        ysb = [sb.tile([P, len(ts), N_OUT], f16, tag=f"ysb{bi}",
                       name=f"ysb{bi}")
               for bi, ts in enumerate(Y_BATCH)]

        ysb12 = [sb.tile([P, N_CHUNK], f16, tag=f"ysb12{h}",
                         name=f"ysb12{h}") for h in range(2)]

        def mm(ps, t, k, nj):
            ci, kr, coff = tile2chunk[(t, k)]
            n0, n1 = nj * N_CHUNK, (nj + 1) * N_CHUNK
            w_ap = (wt[k][:, n0:n1] if k < 3
                    else wt345[:, k - 3, n0:n1])
            nc.tensor.matmul(ps[:, nj, :N_CHUNK],
                             xt[ci][:, kr, coff:coff + P], w_ap,
                             start=(k == 0), stop=(k == K_TILES - 1))

        for t in range(N_TILES - 1):
            bi, bj = tile2batch[t]
            ps = pp.tile([P, 2, 512], f32, tag="ps", name=f"ps{t}")
            for k in range(K_TILES):
                for nj in range(2):
                    mm(ps, t, k, nj)
            nc.scalar.activation(ysb[bi][:, bj, :], ps[:, :, :N_CHUNK],
                                 gelu)
            if bj == len(Y_BATCH[bi]) - 1:
                nc.sync.dma_start(
                    y_dram[bi][:].rearrange("p (a n) -> p a n",
                                            a=len(Y_BATCH[bi])),
                    ysb[bi][:, :, :])

        # Last tile: n-halves finish (and store) independently so the
        # final activation+store chain is half as long.
        ps = pp.tile([P, 2, 512], f32, tag="ps", name="ps12")
        for nj in range(2):
            for k in range(K_TILES):
                mm(ps, 12, k, nj)
            nc.scalar.activation(ysb12[nj][:, :], ps[:, nj, :N_CHUNK],
                                 gelu)
            nc.sync.dma_start(y12_dram[nj][:, :], ysb12[nj][:, :])

        # keep PE/NX busy into the final barrier -> warm NRT postamble
        for _ in range(N_TAILWARM_MM):
            nc.tensor.matmul(zps[:], zt[:, :8], zt[:, 8:520],
                             start=True, stop=True)

    nc.compile()
    return nc


def _get_nc():
    if "nc" not in _CACHE:
        _CACHE["nc"] = _build_nc()
    return _CACHE["nc"]


def _make_in_maps(hidden_states, w_q, w_k, w_v):
    x = np.asarray(hidden_states, dtype=np.float32).reshape(B * S, HIDDEN)
    xT16 = np.ascontiguousarray(x.T).astype(np.float16)     # [768, 12608]
    wcat = np.concatenate(
        [np.asarray(w_q, np.float32), np.asarray(w_k, np.float32),
         np.asarray(w_v, np.float32)], axis=1).astype(np.float16)

    w345 = np.ascontiguousarray(
        np.stack([wcat[k * P:(k + 1) * P, :] for k in (3, 4, 5)],
                 axis=1).reshape(P, 3 * N_OUT))

    in_maps = []
    for c in range(N_CORES):
        base = c * M_PER_CORE
        m = {f"w{k}": np.ascontiguousarray(wcat[k * P:(k + 1) * P, :])
             for k in range(3)}
        m["w345"] = w345
        for ci, (ts, klo, khi) in enumerate(X_CHUNKS):
            segs = []
            for t in ts:
                seg = xT16[:, base + TILE_OFF[t]:base + TILE_OFF[t] + P]
                segs.append(seg.reshape(K_TILES, P, P)[klo:khi]
                            .transpose(1, 0, 2))
            arr = np.concatenate(segs, axis=2)   # [P, khi-klo, csz]
            m[f"x{ci}"] = np.ascontiguousarray(
                arr.reshape(P, (khi - klo) * P * len(ts)))
        in_maps.append(m)
    return in_maps


def _postprocess(results):
    y_full = np.empty((B * S, N_OUT), dtype=np.float32)
    for c in range(N_CORES):
        base = c * M_PER_CORE
        res = results[c]
        for bi, ts in enumerate(Y_BATCH):
            buf = res[f"y{bi}"].reshape(P, len(ts), N_OUT)
            for j, t in enumerate(ts):
                off = base + TILE_OFF[t]
                y_full[off:off + P, :] = buf[:, j, :]
        off = base + TILE_OFF[12]
        y_full[off:off + P, :N_CHUNK] = res["y120"]
        y_full[off:off + P, N_CHUNK:] = res["y121"]
    y_full = y_full.reshape(B, S, N_OUT)
    q = np.ascontiguousarray(y_full[:, :, :RANK])
    k = np.ascontiguousarray(y_full[:, :, RANK:2 * RANK])
    v = np.ascontiguousarray(y_full[:, :, 2 * RANK:])
    return (q, k, v)


def kernel(hidden_states, w_q, w_k, w_v):
    from concourse.bass_utils import run_bass_kernel_spmd

    nc = _get_nc()
    in_maps = _make_in_maps(hidden_states, w_q, w_k, w_v)
    res = run_bass_kernel_spmd(nc, in_maps, list(range(N_CORES)))
    return _postprocess(res.results)
